# revision 17
# baseline (speedup 1.0000x reference)
"""Trainium2 Bass kernel for the ConstraintCRF loss.

Math
----
loss = sum_b (num[b] - den[b]),  den[b] = logsumexp over tag paths.
With G_t = E diag(x_t)  (E = exp(transitions), x_t = exp(logit_t)):

  den = v_0^T G_1 G_2 ... G_511 e,   v_0 = exp(start) * x_0, e = exp(end)

Products of positive matrices contract to rank-1 exponentially fast
(Birkhoff contraction ~0.27 per E application here), so any >=16-step
segment operator M_s is numerically rank-1:  M_s ~= (M_s 1)(1^T M_s)/c_s.
Split t into S=16 segments of L=32 x-slices and replace every interior
operator with its rank-1 sketch:

  den = prod_{s=1..15} (q_s . w_{s+1}) / prod_{s=2..15} c_s
  q_s = E^T a_s,  a_s^T = (v_0 or 1)^T M_s   (forward probe scan)
  w-chain:  E w_s = M_s (1 or e)             (backward probe scan)
  c_s = 1^T M_s 1 = colsum(E) . w_s

Each probe is an independent 32-step scan; each core interleaves 15 of
them (one batch group of 32 rows), so the serial recurrence chain drops
from 256 steps to 32 and the kernel is throughput-bound, not
latency-bound.  E is pre-scaled by 2^-9 so states slowly decay instead
of overflowing: no renormalization at all (the host adds back
511*9*ln2).  bf16 states/weights, fp32 PSUM; junctions + numerator in
host float64.

Layout tricks:
- (fwd, bwd) probe pairs cover the same segment and share one staged x
  block; the bwd stream reads it through a reversed-t access pattern.
  Halves x DMA traffic and on-chip exp work.
- Streams advance in 2 lockstep groups BY DIRECTION (8 fwd-reading / 7
  bwd-reading) sharing one PSUM tile per group, so each wave costs one
  Vector tensor_tensor per group (~170ns fixed PSUM-read cost dominates
  Vector ops; GPSIMD cannot read PSUM on TRN2), and each group's x
  slices form one regular strided AP.
- The initial state (init_vec * first x slice) is premultiplied on the
  host into a tiny `xinit` tensor that wave-1 matmuls read directly --
  no on-chip init ops at all.
- The per-core leftover probe (a_1 on cores 0-3, w_16 on cores 4-7)
  gets its own x slot, host-pre-reversed for w_16 so the program is
  direction-agnostic (its stationary/init are inputs like everything
  else).
"""

import os
import sys
from contextlib import ExitStack

import numpy as np

for _p in ("/opt/trn_rl_repo",):
    if os.path.isdir(_p) and _p not in sys.path:
        sys.path.insert(0, _p)

import concourse.bass as bass
import concourse.tile as tile
from concourse import mybir
from concourse.bass_utils import run_bass_kernel_spmd

try:
    import ml_dtypes

    BF16_NP = ml_dtypes.bfloat16
except ImportError:  # pragma: no cover
    BF16_NP = None

B, T, K = 128, 512, 256
NCORES = 8
NGROUP = 4
NB = B // NGROUP    # 32 batch rows per group (each core carries one group)
S = 16              # segments
LSEG = T // S       # 32 x-slices per stream
NST = S - 1         # 15 streams per core
NSLOT = 8           # staged x blocks (1 singleton + 7 shared pairs)
TC = 4              # t-chunk for exp pipelining
NQ = 4              # x DMA quarters
SCALE_LOG2 = 9      # E pre-scaled by 2^-9 on host

FP32 = mybir.dt.float32
BF16 = mybir.dt.bfloat16

# group 0: slot j read forward (j=0 singleton + 7 pair-fwd probes)
# group 1: slot j+1 read backward (7 pair-bwd probes)
G0, G1 = 8, 7

_compiled = {}

# kept for test.py introspection (exec time / traces)
LAST_RESULTS = None


def _build_nc():
    nc = bass.Bass()

    # x blocks (logits), bf16: [kpart, slot, kchunk, t, b]
    lraw_d = nc.dram_tensor("lraw", [128, NSLOT, 2, LSEG, NB], BF16,
                            kind="ExternalInput")
    # stationary sets (0=fwd E', 1=bwd E'^T, 2=singleton's own)
    temat_d = nc.dram_tensor("temat", [128, 3, 2, K], BF16,
                             kind="ExternalInput")
    # host-premultiplied initial states: [kpart, stream, kchunk, b]
    xinit_d = nc.dram_tensor("xinit", [128, NST, 2, NB], BF16,
                             kind="ExternalInput")
    # outputs, contiguous with the SBUF group tiles (strided layouts blow
    # up into thousands of 64B DMA packets): g0 -> [kpart, state|q,
    # stream, kchunk, b]; g1 only needs its final states (w probes)
    qwout0_d = nc.dram_tensor("qwout0", [128, 2, G0, 2, NB], BF16,
                              kind="ExternalOutput")
    qwout1_d = nc.dram_tensor("qwout1", [128, G1, 2, NB], BF16,
                              kind="ExternalOutput")

    TQ = LSEG // NQ          # 8 t-slices per DMA quarter
    nchunks = LSEG // TC     # 8 exp chunks

    with tile.TileContext(nc) as tc, ExitStack() as ctx:
        const = ctx.enter_context(tc.tile_pool(name="const", bufs=1))
        lstage = ctx.enter_context(tc.tile_pool(name="lstage", bufs=1))
        xbp = ctx.enter_context(tc.tile_pool(name="xb", bufs=1))
        outp = ctx.enter_context(tc.tile_pool(name="outp", bufs=1))
        vps = [
            ctx.enter_context(tc.tile_pool(name=f"v{gi}", bufs=3))
            for gi in range(2)
        ]
        psp = ctx.enter_context(tc.tile_pool(name="ps", bufs=1, space="PSUM"))

        # ---- input staging ---------------------------------------------
        # x quarters ordered [0, 3, 1, 2]: wave 1 needs exp chunks 0 (fwd
        # t=1) and 7 (bwd t=30), i.e. quarters 0 and 3, first.
        lq = [None] * NQ

        def emit_quarter(q):
            st = lstage.tile([128, NSLOT, 2, TQ, NB], BF16, tag=f"ls{q}")
            nc.sync.dma_start(st[:], lraw_d[:, :, :, q * TQ : (q + 1) * TQ, :])
            lq[q] = st

        emit_quarter(0)
        tem = const.tile([128, 3, 2, K], BF16, tag="tem")
        nc.sync.dma_start(tem[:], temat_d[:])
        xi = const.tile([128, NST, 2, NB], BF16, tag="xi")
        nc.sync.dma_start(xi[:], xinit_d[:])
        emit_quarter(3)
        emit_quarter(1)
        emit_quarter(2)

        # exp chunks, ordered outside-in to match fwd/bwd consumption
        xb_t = [None] * nchunks

        def emit_exp(ch):
            q, lo = divmod(ch * TC, TQ)
            xb = xbp.tile([128, NSLOT, 2, TC, NB], BF16, tag=f"xb{ch}")
            nc.scalar.activation(
                xb[:], lq[q][:, :, :, lo : lo + TC, :],
                mybir.ActivationFunctionType.Exp,
            )
            xb_t[ch] = xb

        for ch in (0, 7, 1, 6, 2, 5, 3, 4):
            emit_exp(ch)

        # ---- main loop: 15 interleaved scans in 2 lockstep groups ------
        # state_k <- x[t_k(w)] * (lhsT_k^T @ state_k); state after wave 0
        # is the host-premultiplied xinit.
        def flavor(gi, i):
            return 1 if gi == 1 else (2 if i == 0 else 0)

        def ginfo(gi):
            return (G0, 0) if gi == 0 else (G1, 1)  # (count, slot offset)

        states = [None, None]

        def emit_mms(gi, w, dst):
            ng, _ = ginfo(gi)
            for i in range(ng):
                for jc in range(2):
                    for c in range(2):
                        rhs = (
                            xi[:, (gi * G0) + i, c, :] if w == 1
                            else states[gi][:, i, c, :]
                        )
                        nc.tensor.matmul(
                            dst[:, i, jc, :],
                            tem[:, flavor(gi, i), c, 128 * jc : 128 * (jc + 1)],
                            rhs,
                            start=(c == 0),
                            stop=(c == 1),
                        )

        def gx(gi, w):
            # group x slices: one strided AP over the chunk tile
            ng, so = ginfo(gi)
            t = w if gi == 0 else LSEG - 1 - w
            ch, lo = divmod(t, TC)
            return xb_t[ch][:, so : so + ng, :, lo, :]

        for w in range(1, LSEG):
            for gi in range(2):
                ng, _ = ginfo(gi)
                ps = psp.tile([128, ng, 2, NB], FP32, tag=f"ps{gi}")
                emit_mms(gi, w, ps)
                vn = vps[gi].tile([128, ng, 2, NB], BF16, tag=f"v{gi}")
                nc.vector.tensor_tensor(
                    vn[:], ps[:], gx(gi, w), mybir.AluOpType.mult
                )
                states[gi] = vn

        # ---- junction: q = E'^T a (emission-free step) + outputs -------
        # only g0 needs the junction (fwd probes report q; the singleton's
        # final state is also in g0); g1 reports final states only
        qs = psp.tile([128, G0, 2, NB], FP32, tag="ps0")
        emit_mms(0, LSEG, qs)
        qb = outp.tile([128, G0, 2, NB], BF16, tag="qb0")
        nc.vector.tensor_copy(qb[:], qs[:])
        nc.sync.dma_start(qwout0_d[:, 0], states[0][:])
        nc.sync.dma_start(qwout0_d[:, 1], qb[:])
        nc.sync.dma_start(qwout1_d[:], states[1][:])

    import bass_rust

    bass_rust.move_matmul_waits_to_ldweights(nc.m)
    bass_rust.generate_event_semaphores(nc)
    return nc


def _get_nc():
    if "nc" not in _compiled:
        _compiled["nc"] = _build_nc()
    return _compiled["nc"]


# ---- host-side stream/segment layout ----------------------------------
# Segment s covers x indices [(s-1)*32, s*32).  Cores 0-3 (A) carry the
# even segments + a_1; cores 4-7 (B) the odd segments + w_16.
#   core A slots: 0 -> seg 1 (a_1), j=1..7 -> seg 2j   (a_2j, w_2j)
#   core B slots: 0 -> seg 16 (w_16, pre-reversed), j -> seg 2j+1
# Program stream order: g0 = [singleton, fwd(slot 1..7)], g1 = [bwd(slot
# 1..7)].


def _to_bf16(a):
    assert BF16_NP is not None, "ml_dtypes required for bf16 inputs"
    return np.ascontiguousarray(np.asarray(a, np.float64).astype(np.float32)
                                .astype(BF16_NP))


def _numerator(logits, tags, mask, transitions, start_transitions, end_transitions):
    logits = np.asarray(logits, np.float64)
    tags = np.asarray(tags, np.int64)
    maskf = np.asarray(mask, np.float64)
    b_idx = np.arange(B)
    score = np.asarray(start_transitions, np.float64)[tags[:, 0]]
    trans = np.asarray(transitions, np.float64)[tags[:, :-1], tags[:, 1:]]
    score = score + (trans * maskf[:, 1:]).sum(1)
    emit = np.take_along_axis(logits[:, :-1], tags[:, :-1, None], axis=2)[..., 0]
    score = score + (emit * maskf[:, :-1]).sum(1)
    last_idx = maskf.astype(np.int64).sum(1) - 1
    last_tags = tags[b_idx, last_idx]
    score = score + np.asarray(end_transitions, np.float64)[last_tags]
    score = score + logits[b_idx, -1, last_tags] * maskf[:, -1]
    return score


def _reference_fallback(logits, tags, mask, transitions, start_transitions,
                        end_transitions):
    """Pure-numpy log-space forward algorithm (only used if mask isn't all
    ones, which the staged problem never produces)."""
    lg = np.asarray(logits, np.float64)
    m = np.asarray(mask, bool)
    tr = np.asarray(transitions, np.float64)
    alpha = np.asarray(start_transitions, np.float64)[None, :] + lg[:, 0]
    for t in range(1, T):
        inner = alpha[:, :, None] + tr[None]
        mx = inner.max(1)
        new = np.log(np.exp(inner - mx[:, None, :]).sum(1)) + mx + lg[:, t]
        alpha = np.where(m[:, t][:, None], new, alpha)
    stops = alpha + np.asarray(end_transitions, np.float64)[None, :]
    mx = stops.max(1)
    den = np.log(np.exp(stops - mx[:, None]).sum(1)) + mx
    num = _numerator(logits, tags, mask, transitions, start_transitions,
                     end_transitions)
    return np.float32((num - den).sum())


def _karrange(a):
    """[NB, L, K] -> [128, 2, L, NB] (k-partition-major)."""
    L = a.shape[1]
    return a.transpose(2, 1, 0).reshape(2, 128, L, NB).transpose(1, 0, 2, 3)


def kernel(logits, tags, mask, transitions, start_transitions, end_transitions):
    global LAST_RESULTS
    logits = np.ascontiguousarray(np.asarray(logits, np.float32))
    transitions = np.asarray(transitions, np.float64)
    start_transitions = np.asarray(start_transitions, np.float64)
    end_transitions = np.asarray(end_transitions, np.float64)

    if not np.asarray(mask).all():
        return _reference_fallback(logits, tags, mask, transitions,
                                   start_transitions, end_transitions)

    nc = _get_nc()

    scale = 2.0 ** -SCALE_LOG2
    E = np.exp(transitions) * scale          # f64, scaled
    colsum = E.sum(0)                        # f64 host vector (1^T E')
    te_fwd = E.reshape(2, 128, K).transpose(1, 0, 2)           # [128, 2, K]
    te_bwd = np.ascontiguousarray(E.T).reshape(2, 128, K).transpose(1, 0, 2)

    lg_bf16 = logits.astype(BF16_NP)         # [B, T, K]
    x_bf16 = np.exp(lg_bf16.astype(np.float64))  # f64 of quantized logits

    def seg_x(g, s, rev=False):
        lo = (s - 1) * LSEG
        sl = lg_bf16[g * NB : (g + 1) * NB, lo : lo + LSEG]    # [NB, L, K]
        return sl[:, ::-1] if rev else sl

    in_maps = []
    for core in range(NCORES):
        g = core % NGROUP
        is_a = core < NGROUP
        lr = np.empty((128, NSLOT, 2, LSEG, NB), dtype=BF16_NP)
        xin = np.empty((128, NST, 2, NB), dtype=BF16_NP)

        # slot 0: singleton (a_1 fwd on A; w_16 host-pre-reversed on B)
        lr[:, 0] = _karrange(seg_x(g, 1) if is_a else seg_x(g, S, rev=True))
        segs = [2 * j if is_a else 2 * j + 1 for j in range(1, NSLOT)]
        for j, s in enumerate(segs, start=1):
            lr[:, j] = _karrange(seg_x(g, s))

        # xinit: stream order [singleton, fwd slots 1-7, bwd slots 1-7]
        def xfirst(s, rev, init_vec):
            lo = (s - 1) * LSEG
            t = lo + (LSEG - 1) if rev else lo
            xs = np.exp(np.asarray(lg_bf16[g * NB : (g + 1) * NB, t],
                                   np.float64))               # [NB, K]
            v = (xs * init_vec[None, :]).astype(np.float32).astype(BF16_NP)
            return v.T.reshape(2, 128, NB).transpose(1, 0, 2)  # [128, 2, NB]

        iv_single = (np.exp(start_transitions) if is_a
                     else np.exp(end_transitions))
        xin[:, 0] = xfirst(1 if is_a else S, not is_a, iv_single)
        for j, s in enumerate(segs, start=1):
            xin[:, j] = xfirst(s, False, colsum)               # fwd probes
            xin[:, G0 + j - 1] = xfirst(s, True, np.ones(K))   # bwd probes

        tem = np.empty((128, 3, 2, K), np.float64)
        tem[:, 0] = te_fwd
        tem[:, 1] = te_bwd
        tem[:, 2] = te_fwd if is_a else te_bwd
        in_maps.append({
            "lraw": np.ascontiguousarray(lr),
            "temat": _to_bf16(tem),
            "xinit": np.ascontiguousarray(xin),
        })

    res = run_bass_kernel_spmd(
        nc, in_maps, list(range(NCORES)),
        trace=bool(os.environ.get("CRF_TRACE")),
    )
    LAST_RESULTS = res
    outs = res.results

    # ---- host junctions (float64) ----------------------------------
    den = np.empty(B, np.float64)
    for g in range(NGROUP):
        qv = {}   # q_s = E'^T a_s
        wv = {}   # w_s (E' w_s = M_s b-init)
        for half in (0, 1):
            core = half * NGROUP + g
            is_a = half == 0
            q0 = np.asarray(outs[core]["qwout0"], np.float64)
            q1 = np.asarray(outs[core]["qwout1"], np.float64)

            def vec(arr, k):
                return arr[:, k].transpose(1, 0, 2).reshape(K, NB)

            if is_a:
                qv[1] = vec(q0[:, 1], 0)
            else:
                wv[S] = vec(q0[:, 0], 0)
            segs = [2 * j if is_a else 2 * j + 1 for j in range(1, NSLOT)]
            for j, s in enumerate(segs, start=1):
                qv[s] = vec(q0[:, 1], j)
                wv[s] = vec(q1, j - 1)
        dg = np.zeros(NB, np.float64)
        for s in range(1, S):
            dg += np.log((qv[s] * wv[s + 1]).sum(0))
        for s in range(2, S):
            dg -= np.log(colsum @ wv[s])
        dg += 511.0 * SCALE_LOG2 * np.log(2.0)
        den[g * NB : (g + 1) * NB] = dg

    num = _numerator(logits, tags, mask, transitions, start_transitions,
                     end_transitions)
    return np.float32((num - den).sum())
